# revision 15
# baseline (speedup 1.0000x reference)
"""Trainium2 Bass kernel for nn_CrossAggregator (gnn_message_passing).

out[g,o] = self[g]·W1[o,:] + ea_g^T A_o eb_g,  g=(b,m), A_o = W[o,128:].reshape(128,128)
ea/eb = masked means over 32 neighbors (t=0 / t=1).

Design (per core, batch/8 data-parallel, G=512 rows), phase-rotation scheme:
- All DMA-heavy operands shipped bf16 (halves HBM traffic); PSUM accum stays f32.
- eb-side: mask-mult on DVE (bf16 2x/4x) + masked-mean via K=128 bf16 matmuls
  with a banded selector (BIG) -> ebT [j,g] in PSUM; then 4 partition-group
  rotations of ebT (ebRot_q[p] = ebT[32*((p//32+q)%4) + p%32]) computed once
  per iteration via permutation-stationary matmuls -> ebRotAll [128, 4G] bf16.
- ea-side: ONE K=128 matmul per slab ig with a block-diagonal selector (SEL)
  gives the GROUPED broadcast rep4[p,g] = ea[g, 4ig + p//32] (4 means per
  213ns matmul instead of 4 separate K=32 broadcasts -> 4x less PE).
- pt[p,(q,g)] = ebRotAll[p,(q,g)] * rep4[p,g] on DVE (all-bf16 SBUF); phase q
  pairs j-chunk (p//32+q)%4 with i-group p//32, so the 4 phases of a slab
  cover all (i,j) pairs exactly once.
- main contraction on PE: p_out[o,g] += W2Q[ig,q]^T @ pt_q  (bf16 in, f32 acc),
  W2Q[ig,q][p,o] = W[o, 128 + (4ig+p//32)*128 + jrot_q(p)] packed on host.
- main loop pipelined in two-slab steps (2 rep + 8 main matmuls per step, depth
  PIPE=2) so the tensor engine runs in long uninterrupted bursts (full pstate).
- half the nb-mask multiplies route to the otherwise-idle Pool engine (0.42
  efficiency, but prefetched across the previous iteration's main phase).
- host does only layout transforms (shard/permute/pack/cast) + output unshard.
"""
import sys
import numpy as np

for _p in ("/opt/trn_rl_repo", "/root/.axon_site/_ro/trn_rl_repo"):
    if _p not in sys.path:
        sys.path.insert(0, _p)

B, M, TWO, NN, D = 1024, 4, 2, 32, 128
OUT = 128
NCORES = 8
BC = B // NCORES          # batches per core
G = BC * M                # 512 rows per core
NIG = D // 4              # 32 slabs of 4 i's (partition dim packs (isub, n))

_CACHE = {}


def _build_nc():
    import os
    import concourse.bacc as bacc_mod
    import concourse.mybir as mybir
    from concourse.tile import TileContext

    F32 = mybir.dt.float32
    BF16 = mybir.dt.bfloat16
    MUL = mybir.AluOpType.mult

    nc = bacc_mod.Bacc(None)

    PIPE = int(os.environ.get("PIPE", "2"))   # pair-step pipeline depth (PSUM-limited)
    NBPOOL = int(os.environ.get("NBPOOL", "4"))  # of 8 nb-mask ops routed to Pool

    d_naP = nc.declare_dram_parameter("naP", [16, 128, 2 * G], BF16, isOutput=False)
    d_nbQ = nc.declare_dram_parameter("nbQ", [8, 128, 4 * G], BF16, isOutput=False)
    d_maskA = nc.declare_dram_parameter("maskA", [128, G], BF16, isOutput=False)
    d_maskB = nc.declare_dram_parameter("maskB", [128, G], BF16, isOutput=False)
    d_selfT = nc.declare_dram_parameter("selfT", [D, G], BF16, isOutput=False)
    d_W1 = nc.declare_dram_parameter("W1a", [D, OUT], BF16, isOutput=False)
    d_W2 = nc.declare_dram_parameter("W2Q", [128, NIG * 4 * OUT], BF16, isOutput=False)
    d_BIG = nc.declare_dram_parameter("BIG", [128, 252], BF16, isOutput=False)
    d_SEL = nc.declare_dram_parameter("SEL", [128, 128], BF16, isOutput=False)
    d_PERM = nc.declare_dram_parameter("PERM", [128, 4 * 128], BF16, isOutput=False)
    d_out = nc.declare_dram_parameter("outT", [OUT, G], F32, isOutput=True)

    with TileContext(nc) as tc:
        with (
            tc.tile_pool(name="const", bufs=1) as cpool,
            tc.tile_pool(name="nb_raw", bufs=3) as nbpool,
            tc.tile_pool(name="nb_msk", bufs=5) as nbmpool,
            tc.tile_pool(name="na_raw", bufs=3) as napool,
            tc.tile_pool(name="na_msk", bufs=3) as nampool,
            tc.tile_pool(name="repsb", bufs=3) as rspool,
            tc.tile_pool(name="pt", bufs=3) as ptpool,
            tc.tile_pool(name="ebt", bufs=2) as ebpool,
            tc.tile_pool(name="ebrot", bufs=2) as erpool,
            tc.tile_pool(name="misc", bufs=1) as mpool,
            tc.tile_pool(name="ps_ebt", bufs=1, space="PSUM") as ps_ebt,
            tc.tile_pool(name="ps_rot", bufs=1, space="PSUM") as ps_rot,
            tc.tile_pool(name="ps_rep", bufs=PIPE, space="PSUM") as ps_rep,
            tc.tile_pool(name="ps_out", bufs=1, space="PSUM") as ps_out,
        ):
            # constants + weights (resident in SBUF across loop iterations)
            big_t = cpool.tile([128, 252], BF16, tag="big")
            nc.sync.dma_start(out=big_t[:], in_=d_BIG[:])
            sel_t = cpool.tile([128, 128], BF16, tag="sel")
            nc.sync.dma_start(out=sel_t[:], in_=d_SEL[:])
            perm_t = cpool.tile([128, 4 * 128], BF16, tag="perm")
            nc.sync.dma_start(out=perm_t[:], in_=d_PERM[:])
            maskA_t = cpool.tile([128, G], BF16, tag="ma")
            nc.sync.dma_start(out=maskA_t[:], in_=d_maskA[:])
            maskB_t = cpool.tile([128, G], BF16, tag="mb")
            nc.sync.dma_start(out=maskB_t[:], in_=d_maskB[:])
            selfT_t = cpool.tile([D, G], BF16, tag="sT")
            nc.sync.dma_start(out=selfT_t[:], in_=d_selfT[:])
            w1_t = cpool.tile([D, OUT], BF16, tag="w1")
            nc.sync.dma_start(out=w1_t[:], in_=d_W1[:])
            w2_t = cpool.tile([128, NIG * 4 * OUT], BF16, tag="w2")
            nc.sync.dma_start(out=w2_t[:], in_=d_W2[:])

            _loop_n = int(os.environ.get("KERNEL_LOOP", "0"))
            from contextlib import nullcontext
            _ctx = tc.For_i(0, _loop_n, 1) if _loop_n else nullcontext()
            with _ctx:
                # ---- EB phase: ebT[j, g] in PSUM ----
                p_ebt = ps_ebt.tile([128, G], F32, tag="ebt")
                for sg in range(8):  # 4 slabs per DMA
                    nb4 = nbpool.tile([128, 4 * G], BF16, tag="nb4")
                    nc.sync.dma_start(out=nb4[:], in_=d_nbQ[sg])
                    mb4 = nbmpool.tile([128, 4 * G], BF16, tag="mb4")
                    # Pool is idle otherwise; route every other op there so DVE
                    # keeps headroom for the pt multiplies (Pool runs ahead
                    # during the previous iteration's main phase).
                    eng = nc.gpsimd if (sg % 2 == 0 and sg // 2 < NBPOOL) else nc.vector
                    eng.tensor_tensor(
                        out=mb4[:].rearrange("p (s c) -> p s c", s=4),
                        in0=nb4[:].rearrange("p (s c) -> p s c", s=4),
                        in1=maskB_t[:][:, None, :].broadcast_to([128, 4, G]),
                        op=MUL,
                    )
                    for u in range(4):
                        jg = 4 * sg + u
                        nc.tensor.matmul(
                            p_ebt[:],
                            big_t[:, 124 - 4 * jg : 252 - 4 * jg],
                            mb4[:, G * u : G * (u + 1)],
                            start=(jg == 0),
                            stop=(jg == NIG - 1),
                        )
                ebT_sb = ebpool.tile([128, G], BF16, tag="ebsb")
                nc.scalar.copy(out=ebT_sb[:], in_=p_ebt[:])

                # ---- MAIN phase (software-pipelined over slabs; 4 i's per slab) ----
                ma2_tiles = {}
                rep_tiles = {}
                pt_tiles = {}
                ebRot = erpool.tile([128, 4 * G], BF16, tag="ebrot")

                NS = NIG // 2  # 16 pair-steps; step s covers slabs (2s, 2s+1)

                def emit_load(s):
                    # load+mask the na pair for step s
                    na2 = napool.tile([128, 2 * G], BF16, tag="na2")
                    nc.sync.dma_start(out=na2[:], in_=d_naP[s])
                    ma2 = nampool.tile([128, 2 * G], BF16, tag="ma2")
                    nc.vector.tensor_tensor(
                        out=ma2[:].rearrange("p (s c) -> p s c", s=2),
                        in0=na2[:].rearrange("p (s c) -> p s c", s=2),
                        in1=maskA_t[:][:, None, :].broadcast_to([128, 2, G]),
                        op=MUL,
                    )
                    ma2_tiles[s] = ma2

                def emit_rep(s):
                    # grouped broadcast rep2[p,(t,g)] = ea[g, 4*(2s+t) + p//32]
                    ma2 = ma2_tiles.pop(s)
                    rep = ps_rep.tile([128, 2 * G], F32, tag="rep")
                    for t in range(2):
                        nc.tensor.matmul(
                            rep[:, G * t : G * (t + 1)],
                            sel_t[:],
                            ma2[:, G * t : G * (t + 1)],
                            start=True,
                            stop=True,
                        )
                    rep_tiles[s] = rep

                def emit_pt(s):
                    rep = rep_tiles.pop(s)
                    rep_sb = rspool.tile([128, 2 * G], BF16, tag="repsb")
                    nc.scalar.copy(out=rep_sb[:], in_=rep[:])
                    pt8 = ptpool.tile([128, 8 * G], BF16, tag="pt8")
                    for t in range(2):
                        nc.vector.tensor_tensor(
                            out=pt8[:, 4 * G * t : 4 * G * (t + 1)].rearrange(
                                "p (q c) -> p q c", q=4
                            ),
                            in0=ebRot[:].rearrange("p (q c) -> p q c", q=4),
                            in1=rep_sb[:, G * t : G * (t + 1)][:, None, :]
                            .broadcast_to([128, 4, G]),
                            op=MUL,
                        )
                    pt_tiles[s] = pt8

                def emit_main(s):
                    pt8 = pt_tiles.pop(s)
                    for t in range(2):
                        ig = 2 * s + t
                        for q in range(4):
                            nc.tensor.matmul(
                                p_out[:],
                                w2_t[:, (ig * 4 + q) * OUT : (ig * 4 + q + 1) * OUT],
                                pt8[:, (4 * t + q) * G : (4 * t + q + 1) * G],
                                start=False,
                                stop=(ig == NIG - 1 and q == 3),
                            )

                emit_load(0)
                emit_load(1)
                emit_load(2)
                # early rep keeps PE busy while Act copies ebT for the rotations
                emit_rep(0)

                # ebT partition-group rotations (q=0 identity included), in two
                # PSUM halves to stay within bank budget
                for half in range(2):
                    p_rot = ps_rot.tile([128, 2 * G], F32, tag="rot")
                    for qq in range(2):
                        q = 2 * half + qq
                        nc.tensor.matmul(
                            p_rot[:, G * qq : G * (qq + 1)],
                            perm_t[:, 128 * q : 128 * (q + 1)],
                            ebT_sb[:],
                            start=True,
                            stop=True,
                        )
                    nc.scalar.copy(
                        out=ebRot[:, 2 * G * half : 2 * G * (half + 1)], in_=p_rot[:]
                    )

                p_out = ps_out.tile([OUT, G], F32, tag="out")
                nc.tensor.matmul(p_out[:], w1_t[:], selfT_t[:], start=True, stop=False)

                emit_pt(0)
                for s in range(1, NS + PIPE):
                    if s < NS:
                        if s + 2 < NS:
                            emit_load(s + 2)
                        emit_rep(s)
                        emit_pt(s)
                    if s >= PIPE:
                        emit_main(s - PIPE)

                out_sb = mpool.tile([OUT, G], F32, tag="osb")
                nc.scalar.copy(out=out_sb[:], in_=p_out[:])
                nc.sync.dma_start(out=d_out[:], in_=out_sb[:])

    nc.finalize()
    return nc


def _host_prep(self_vectors, neighbor_vectors, masks, W):
    import ml_dtypes

    f32 = np.float32
    bf16 = ml_dtypes.bfloat16
    sv = np.ascontiguousarray(self_vectors, dtype=f32)
    nv = np.ascontiguousarray(neighbor_vectors, dtype=f32)
    mk = np.ascontiguousarray(masks, dtype=f32)
    Wf = np.ascontiguousarray(W, dtype=f32)

    # per-core packs; partition dim packs (sub, n) where d = 4*slab + sub
    nvc = nv.reshape(NCORES, G, TWO, NN, D)          # [c, g, t, n, d]
    naR = nvc[:, :, 0].transpose(0, 3, 2, 1).reshape(NCORES, NIG, 128, G)
    nbR = nvc[:, :, 1].transpose(0, 3, 2, 1).reshape(NCORES, NIG, 128, G)
    # group slabs so each DMA is contiguous per partition: pairs for na, quads for nb
    naP = np.ascontiguousarray(
        naR.reshape(NCORES, 16, 2, 128, G).transpose(0, 1, 3, 2, 4)
        .reshape(NCORES, 16, 128, 2 * G)
    ).astype(bf16)
    nbQ = np.ascontiguousarray(
        nbR.reshape(NCORES, 8, 4, 128, G).transpose(0, 1, 3, 2, 4)
        .reshape(NCORES, 8, 128, 4 * G)
    ).astype(bf16)
    mkc = mk.reshape(NCORES, G, TWO, NN)             # [c, g, t, n]
    mA = mkc[:, :, 0].transpose(0, 2, 1)             # [c, n, g]
    mB = mkc[:, :, 1].transpose(0, 2, 1)
    maskA = np.ascontiguousarray(
        np.broadcast_to(mA[:, None], (NCORES, 4, NN, G)).reshape(NCORES, 128, G)
    ).astype(bf16)
    maskB = np.ascontiguousarray(
        np.broadcast_to(mB[:, None], (NCORES, 4, NN, G)).reshape(NCORES, 128, G)
    ).astype(bf16)
    selfT = np.ascontiguousarray(
        sv.reshape(NCORES, G, D).transpose(0, 2, 1)
    ).astype(bf16)  # [c, d, g]

    # shared weights
    W1a = np.ascontiguousarray(Wf[:, :D].T).astype(bf16)          # [d, o]
    jio = Wf[:, D:].reshape(OUT, D, D).transpose(2, 1, 0)         # [j, i, o]
    p = np.arange(128)
    c, n = p // 32, p % 32
    # W2Q[ig, q, p, o] = jio[jrot_q(p), 4ig + p//32, o]
    jrot = 32 * ((c[None, :] + np.arange(4)[:, None]) % 4) + n[None, :]   # [4, 128]
    i_idx = 4 * np.arange(NIG)[:, None, None] + c[None, None, :]          # [NIG,1,128]
    W2Q = jio[jrot[None, :, :], i_idx, :]                                 # [NIG,4,128,OUT]
    W2Q = np.ascontiguousarray(
        W2Q.transpose(2, 0, 1, 3).reshape(128, NIG * 4 * OUT)
    ).astype(bf16)
    BIG = np.zeros((128, 252), f32)
    r = np.arange(128)
    BIG[r, 124 + r // 32] = 1.0 / 32.0
    BIG = BIG.astype(bf16)
    SEL = np.where(r[:, None] // 32 == r[None, :] // 32, 1.0 / 32.0, 0.0).astype(bf16)
    PERM = np.zeros((128, 4, 128), f32)
    for q in range(4):
        PERM[jrot[q], q, p] = 1.0
    PERM = PERM.reshape(128, 4 * 128).astype(bf16)

    in_maps = []
    for cc in range(NCORES):
        in_maps.append(
            {
                "naP": naP[cc],
                "nbQ": nbQ[cc],
                "maskA": maskA[cc],
                "maskB": maskB[cc],
                "selfT": selfT[cc],
                "W1a": W1a,
                "W2Q": W2Q,
                "BIG": BIG,
                "SEL": SEL,
                "PERM": PERM,
            }
        )
    return in_maps


def _unshard_out(outT_concat, b):
    """[NCORES*OUT, G] (or [NCORES, OUT, G]) core-major outT -> [B, M, OUT] + b."""
    outT = np.asarray(outT_concat).reshape(NCORES, OUT, G)
    out = np.empty((B, M, OUT), np.float32)
    for c in range(NCORES):
        out[c * BC : (c + 1) * BC] = outT[c].T.reshape(BC, M, OUT)
    out += np.asarray(b, np.float32)[None, None, :]
    return out


def kernel(self_vectors, neighbor_vectors, masks, W, b):
    from concourse.bass_utils import run_bass_kernel_spmd

    if "nc" not in _CACHE:
        _CACHE["nc"] = _build_nc()
    nc = _CACHE["nc"]
    in_maps = _host_prep(self_vectors, neighbor_vectors, masks, W)
    results = run_bass_kernel_spmd(nc, in_maps, list(range(NCORES))).results
    outT = np.stack([results[c]["outT"] for c in range(NCORES)])
    return _unshard_out(outT, b)
